# revision 1
# baseline (speedup 1.0000x reference)
import sys

for _p in ("/opt/trn_rl_repo", "/root/.axon_site/_ro/trn_rl_repo"):
    if _p not in sys.path:
        sys.path.insert(0, _p)

import numpy as np
import ml_dtypes

import concourse.bass as bass
import concourse.mybir as mybir
import concourse.tile as tile
from concourse import bacc
from concourse import bass_utils

BF16 = ml_dtypes.bfloat16

P = 128
B = 8
T = 1024
S0 = 1500
S = 1536
D = 1024
H = 16
Dh = 64
DT = D // P
ST = S // P
NPAIR = H // 2
HW = Dh + 1
SCALE = Dh ** -0.5

f32 = mybir.dt.float32
bf16 = mybir.dt.bfloat16


def build_bass():
    nc = bacc.Bacc("TRN2", target_bir_lowering=False, debug=False,
                   enable_asserts=False, num_devices=B)

    xT_d = nc.dram_tensor("xT", [D, T], bf16, kind="ExternalInput")
    kT_d = nc.dram_tensor("kT", [D, S], bf16, kind="ExternalInput")
    va_d = nc.dram_tensor("vaug", [S, H * HW], bf16, kind="ExternalInput")
    wqT_d = nc.dram_tensor("wqT", [D, D], bf16, kind="ExternalInput")
    bq_d = nc.dram_tensor("bqr", [P, DT], f32, kind="ExternalInput")
    woT_d = nc.dram_tensor("woT", [D, D], bf16, kind="ExternalInput")
    bo_d = nc.dram_tensor("bor", [P, DT], f32, kind="ExternalInput")
    outT_d = nc.dram_tensor("outT", [D, T], f32, kind="ExternalOutput")

    EXP = mybir.ActivationFunctionType.Exp

    with tile.TileContext(nc) as tc:
        with (
            tc.tile_pool(name="const", bufs=1) as cp,
            tc.tile_pool(name="work", bufs=2) as wp,
            tc.tile_pool(name="psum_mm", bufs=2, space="PSUM") as mmp,
            tc.tile_pool(name="psum_pv", bufs=2, space="PSUM") as pvp,
        ):
            def load1(dram, cols, j, tagbase, dt=bf16):
                t = cp.tile([P, cols], dt, name=f"{tagbase}{j}",
                            tag=f"{tagbase}{j}")
                nc.sync.dma_start(t[:], dram[j * P:(j + 1) * P, :])
                return t

            bq_sb = cp.tile([P, DT], f32, name="bq_sb", tag="bq_sb")
            nc.sync.dma_start(bq_sb[:], bq_d[:, :])
            xT_sb = [load1(xT_d, T, j, "xTs") for j in range(DT)]
            kT_sb = {0: load1(kT_d, S, 0, "kTs")}
            va_sb = {0: load1(va_d, H * HW, 0, "vas")}
            wqT_sb = [load1(wqT_d, D, j, "wqTs") for j in range(DT)]
            va_sb.update({c: load1(va_d, H * HW, c, "vas") for c in (1, 2, 3)})
            kT_sb.update({j: load1(kT_d, S, j, "kTs") for j in range(1, DT)})
            va_sb.update({c: load1(va_d, H * HW, c, "vas")
                          for c in range(4, ST)})
            woT_sb = [load1(woT_d, D, j, "woTs") for j in range(DT)]
            bo_sb = cp.tile([P, DT], f32, name="bo_sb", tag="bo_sb")
            nc.sync.dma_start(bo_sb[:], bo_d[:, :])

            qT_sb = [cp.tile([P, T], bf16, name=f"qTs{j}", tag=f"qTs{j}")
                     for j in range(DT)]
            aT_sb = [cp.tile([P, T], bf16, name=f"aTs{j}", tag=f"aTs{j}")
                     for j in range(DT)]

            for j in range(DT):
                ps = mmp.tile([P, T], f32, name=f"qp{j}", tag="mm")
                for dt_i in range(DT):
                    for tch in range(2):
                        tsl = slice(tch * 512, (tch + 1) * 512)
                        nc.tensor.matmul(
                            ps[:, tsl],
                            lhsT=wqT_sb[dt_i][:, j * P:(j + 1) * P],
                            rhs=xT_sb[dt_i][:, tsl],
                            start=(dt_i == 0), stop=(dt_i == DT - 1),
                        )
                nc.vector.tensor_scalar_add(qT_sb[j][:, :], ps[:, :],
                                            bq_sb[:, j:j + 1])

            for j in range(NPAIR):
                pv = [pvp.tile([P, T], f32, name=f"pv{j}_{a}", tag="pv")
                      for a in range(2)]
                for c in range(ST):
                    csl = slice(c * P, (c + 1) * P)
                    sc = [mmp.tile([P, T], f32, name=f"sc{j}_{c}_{a}",
                                   tag="mm") for a in range(2)]
                    for a in range(2):
                        rows = slice(a * Dh, (a + 1) * Dh)
                        for tch in range(2):
                            tsl = slice(tch * 512, (tch + 1) * 512)
                            nc.tensor.matmul(
                                sc[a][:, tsl],
                                lhsT=kT_sb[j][rows, csl],
                                rhs=qT_sb[j][rows, tsl],
                                start=True, stop=True,
                            )
                    pt = [None, None]
                    for a in range(2):
                        pt[a] = wp.tile([P, T], bf16, name=f"pt{j}_{c}_{a}",
                                        tag="pt", bufs=8)
                        nc.scalar.activation(pt[a][:, :], sc[a][:, :], EXP)
                    for a in range(2):
                        h = 2 * j + a
                        for tch in range(2):
                            tsl = slice(tch * 512, (tch + 1) * 512)
                            nc.tensor.matmul(
                                pv[a][0:HW, tsl],
                                lhsT=va_sb[c][:, h * HW:(h + 1) * HW],
                                rhs=pt[a][:, tsl],
                                start=(c == 0), stop=(c == ST - 1),
                            )
                for a in range(2):
                    pvsb = wp.tile([HW, T], f32, name=f"pvsb{j}_{a}",
                                   tag="pvsb", bufs=3)
                    nc.vector.tensor_copy(pvsb[:, :], pv[a][0:HW, :])
                    dsm = wp.tile([Dh, 16], f32, name=f"ds{j}_{a}",
                                  tag="dsm", bufs=4)
                    nc.sync.dma_start(dsm[:, :], pvsb[Dh:Dh + 1, :])
                    nc.vector.reciprocal(dsm[:, :], dsm[:, :])
                    rrow = wp.tile([1, T], f32, name=f"rr{j}_{a}", tag="rrow",
                                   bufs=4)
                    nc.sync.dma_start(rrow[:, :], dsm[:, :])
                    nrm = wp.tile([Dh, T], f32, name=f"nr{j}_{a}", tag="nrm",
                                  bufs=4)
                    nc.gpsimd.partition_broadcast(nrm[:, :], rrow[0:1, :])
                    nc.vector.tensor_mul(
                        aT_sb[j][a * Dh:(a + 1) * Dh, :],
                        pvsb[0:Dh, :], nrm[:, :])

            for fj in range(DT):
                ps = mmp.tile([P, T], f32, name=f"op{fj}", tag="mm")
                for et in range(DT):
                    for tch in range(2):
                        tsl = slice(tch * 512, (tch + 1) * 512)
                        nc.tensor.matmul(
                            ps[:, tsl],
                            lhsT=woT_sb[et][:, fj * P:(fj + 1) * P],
                            rhs=aT_sb[et][:, tsl],
                            start=(et == 0), stop=(et == DT - 1),
                        )
                ost = wp.tile([P, T], f32, name=f"ost{fj}", tag="ost", bufs=3)
                nc.vector.tensor_scalar_add(ost[:, :], ps[:, :],
                                            bo_sb[:, fj:fj + 1])
                nc.sync.dma_start(outT_d[fj * P:(fj + 1) * P, :], ost[:, :])

    nc.compile()
    return nc


def prep_inputs(x, k, v, wq, bq, wo, bo):
    x = np.asarray(x, np.float32)
    k = np.asarray(k, np.float32)
    v = np.asarray(v, np.float32)
    wq = np.asarray(wq, np.float32)
    bq = np.asarray(bq, np.float32)
    wo = np.asarray(wo, np.float32)
    bo = np.asarray(bo, np.float32)

    wqT = np.ascontiguousarray((wq * SCALE).T).astype(BF16)
    bqr = np.ascontiguousarray((bq * SCALE).reshape(DT, P).T)
    woT = np.ascontiguousarray(wo.T).astype(BF16)
    bor = np.ascontiguousarray(bo.reshape(DT, P).T)

    in_maps = []
    for b in range(x.shape[0]):
        xT = np.ascontiguousarray(x[b].T).astype(BF16)
        kT = np.zeros((D, S), BF16)
        kT[:, :S0] = k[b].T.astype(BF16)
        vaug = np.zeros((S, H * HW), BF16)
        vb = v[b].astype(BF16)
        for h in range(H):
            vaug[:S0, h * HW:h * HW + Dh] = vb[:, h * Dh:(h + 1) * Dh]
            vaug[:S0, h * HW + Dh] = BF16(1.0)
        in_maps.append({
            "xT": xT, "kT": kT, "vaug": np.ascontiguousarray(vaug),
            "wqT": wqT, "bqr": bqr, "woT": woT, "bor": bor,
        })
    return in_maps


_NC_CACHE = {}


def kernel(x, k, v, wq, bq, wo, bo, _trace=False):
    if "nc" not in _NC_CACHE:
        _NC_CACHE["nc"] = build_bass()
    nc = _NC_CACHE["nc"]
    in_maps = prep_inputs(x, k, v, wq, bq, wo, bo)
    res = bass_utils.run_bass_kernel_spmd(
        nc, in_maps, core_ids=list(range(B)), trace=_trace)
    _NC_CACHE["last_result"] = res
    out = np.stack([np.ascontiguousarray(r["outT"].T) for r in res.results])
    return out



# revision 2
# speedup vs baseline: 1.1797x; 1.1797x over previous
import sys

for _p in ("/opt/trn_rl_repo", "/root/.axon_site/_ro/trn_rl_repo"):
    if _p not in sys.path:
        sys.path.insert(0, _p)

import numpy as np
import ml_dtypes

import concourse.bass as bass
import concourse.mybir as mybir
import concourse.tile as tile
from concourse import bacc
from concourse import bass_utils

BF16 = ml_dtypes.bfloat16

P = 128
B = 8
T = 1024
S0 = 1500
S = 1536
D = 1024
H = 16
Dh = 64
DT = D // P
ST = S // P
NPAIR = H // 2
HW = Dh + 1
SCALE = Dh ** -0.5

f32 = mybir.dt.float32
bf16 = mybir.dt.bfloat16


def build_bass():
    nc = bacc.Bacc("TRN2", target_bir_lowering=False, debug=False,
                   enable_asserts=False, num_devices=B)

    xT_d = nc.dram_tensor("xT", [D, T], bf16, kind="ExternalInput")
    kT_d = nc.dram_tensor("kT", [D, S], bf16, kind="ExternalInput")
    va_d = nc.dram_tensor("vaug", [S, H * HW], bf16, kind="ExternalInput")
    wqT_d = nc.dram_tensor("wqT", [D, D], bf16, kind="ExternalInput")
    bq_d = nc.dram_tensor("bqr", [P, DT], f32, kind="ExternalInput")
    woT_d = nc.dram_tensor("woT", [D, D], bf16, kind="ExternalInput")
    bo_d = nc.dram_tensor("bor", [P, DT], f32, kind="ExternalInput")
    outT_d = nc.dram_tensor("outT", [D, T], f32, kind="ExternalOutput")

    EXP = mybir.ActivationFunctionType.Exp

    with tile.TileContext(nc) as tc:
        with (
            tc.tile_pool(name="const", bufs=1) as cp,
            tc.tile_pool(name="work", bufs=2) as wp,
            tc.tile_pool(name="psum_sc", bufs=2, space="PSUM") as scp,
            tc.tile_pool(name="psum_pv", bufs=3, space="PSUM") as pvp,
            tc.tile_pool(name="psum_qp", bufs=1, space="PSUM") as qpp,
        ):
            def load1(dram, cols, j, tagbase, eng=nc.sync, dt=bf16):
                t = cp.tile([P, cols], dt, name=f"{tagbase}{j}",
                            tag=f"{tagbase}{j}")
                eng.dma_start(t[:], dram[j * P:(j + 1) * P, :])
                return t

            xT_sb = [load1(xT_d, T, j, "xTs") for j in range(DT)]
            wqT_sb = [load1(wqT_d, D, j, "wqTs", eng=nc.scalar)
                      for j in range(DT)]
            bq_sb = cp.tile([P, DT], f32, name="bq_sb", tag="bq_sb")
            nc.scalar.dma_start(bq_sb[:], bq_d[:, :])
            kT_sb = {0: load1(kT_d, S, 0, "kTs")}
            va_sb = {c: load1(va_d, H * HW, c, "vas") for c in range(4)}
            kT_sb.update({j: load1(kT_d, S, j, "kTs") for j in range(1, DT)})
            va_sb.update({c: load1(va_d, H * HW, c, "vas")
                          for c in range(4, ST)})
            woT_sb = [load1(woT_d, D, j, "woTs") for j in range(DT)]
            bo_sb = cp.tile([P, DT], f32, name="bo_sb", tag="bo_sb")
            nc.sync.dma_start(bo_sb[:], bo_d[:, :])

            qT_sb = [cp.tile([P, T], bf16, name=f"qTs{j}", tag=f"qTs{j}")
                     for j in range(DT)]
            aT_sb = [cp.tile([P, T], bf16, name=f"aTs{j}", tag=f"aTs{j}")
                     for j in range(DT)]

            def q_chain_ops(j, tch):
                tsl = slice(tch * 512, (tch + 1) * 512)
                ps = qpp.tile([P, 512], f32, name=f"qp{j}_{tch}", tag="qp")
                ops = []
                for dt_i in range(DT):
                    def mm(dt_i=dt_i, ps=ps, tsl=tsl):
                        nc.tensor.matmul(
                            ps[:, :],
                            lhsT=wqT_sb[dt_i][:, j * P:(j + 1) * P],
                            rhs=xT_sb[dt_i][:, tsl],
                            start=(dt_i == 0), stop=(dt_i == DT - 1),
                        )
                    ops.append(mm)

                def evict(ps=ps, tsl=tsl, j=j):
                    nc.vector.tensor_scalar_add(qT_sb[j][:, tsl], ps[:, :],
                                                bq_sb[:, j:j + 1])
                ops.append(evict)
                return ops

            for tch in range(2):
                for op in q_chain_ops(0, tch):
                    op()

            qops = {j: q_chain_ops(j, 0) + q_chain_ops(j, 1)
                    for j in range(1, DT)}

            halves = [(j, th) for j in range(NPAIR) for th in range(2)]
            steps = [(h, c) for h in range(len(halves)) for c in range(ST)]
            NSTEP = len(steps)

            sc_t = [None] * NSTEP
            pt_t = [None] * NSTEP
            pv_t = {}

            def emit_sc(i):
                h, c = steps[i]
                j, th = halves[h]
                tsl = slice(th * 512, (th + 1) * 512)
                csl = slice(c * P, (c + 1) * P)
                sc = scp.tile([P, T], f32, name=f"sc{i}", tag="sc")
                sc_t[i] = sc
                for a in range(2):
                    rows = slice(a * Dh, (a + 1) * Dh)
                    nc.tensor.matmul(
                        sc[:, a * 512:(a + 1) * 512],
                        lhsT=kT_sb[j][rows, csl],
                        rhs=qT_sb[j][rows, tsl],
                        start=True, stop=True,
                    )

            def emit_exp(i):
                pt = wp.tile([P, T], bf16, name=f"pt{i}", tag="pt", bufs=4)
                pt_t[i] = pt
                nc.scalar.activation(pt[:, :], sc_t[i][:, :], EXP)

            def emit_pv(i):
                h, c = steps[i]
                j, th = halves[h]
                if c == 0:
                    pv_t[h] = [pvp.tile([HW, 512], f32, name=f"pv{h}_{a}",
                                        tag="pv") for a in range(2)]
                for a in range(2):
                    hh = 2 * j + a
                    nc.tensor.matmul(
                        pv_t[h][a][0:HW, :],
                        lhsT=va_sb[c][:, hh * HW:(hh + 1) * HW],
                        rhs=pt_t[i][:, a * 512:(a + 1) * 512],
                        start=(c == 0), stop=(c == ST - 1),
                    )

            def emit_norm(h):
                j, th = halves[h]
                tsl = slice(th * 512, (th + 1) * 512)
                for a in range(2):
                    pvsb = wp.tile([HW, 512], f32, name=f"pvsb{h}_{a}",
                                   tag="pvsb", bufs=4)
                    nc.vector.tensor_copy(pvsb[:, :], pv_t[h][a][0:HW, :])
                    dsm = wp.tile([Dh, 8], f32, name=f"ds{h}_{a}",
                                  tag="dsm", bufs=4)
                    nc.sync.dma_start(dsm[:, :], pvsb[Dh:Dh + 1, :])
                    nc.vector.reciprocal(dsm[:, :], dsm[:, :])
                    rrow = wp.tile([1, 512], f32, name=f"rr{h}_{a}",
                                   tag="rrow", bufs=4)
                    nc.sync.dma_start(rrow[:, :], dsm[:, :])
                    nrm = wp.tile([Dh, 512], f32, name=f"nr{h}_{a}",
                                  tag="nrm", bufs=4)
                    nc.gpsimd.partition_broadcast(nrm[:, :], rrow[0:1, :])
                    nc.vector.tensor_mul(
                        aT_sb[j][a * Dh:(a + 1) * Dh, tsl],
                        pvsb[0:Dh, :], nrm[:, :])

            def q_budget(i):
                h, c = steps[i]
                j, th = halves[h]
                if th != 0 or (j + 1) not in qops:
                    return 0
                return 2

            emit_sc(0)
            for i in range(NSTEP):
                emit_exp(i)
                if i + 1 < NSTEP:
                    emit_sc(i + 1)
                if i >= 1:
                    emit_pv(i - 1)
                    ph, pc = steps[i - 1]
                    if pc == ST - 1:
                        emit_norm(ph)
                h, c = steps[i]
                j, th = halves[h]
                tgt = j + 1
                if th == 0 and tgt in qops:
                    pend = qops[tgt]
                    for _ in range(min(2, len(pend))):
                        pend.pop(0)()
                    if not pend:
                        del qops[tgt]
            emit_pv(NSTEP - 1)
            emit_norm(len(halves) - 1)

            for fj in range(DT):
                for tch in range(2):
                    tsl = slice(tch * 512, (tch + 1) * 512)
                    ps = scp.tile([P, 512], f32, name=f"op{fj}_{tch}",
                                  tag="sc")
                    for et in range(DT):
                        nc.tensor.matmul(
                            ps[:, :],
                            lhsT=woT_sb[et][:, fj * P:(fj + 1) * P],
                            rhs=aT_sb[et][:, tsl],
                            start=(et == 0), stop=(et == DT - 1),
                        )
                    ost = wp.tile([P, 512], f32, name=f"ost{fj}_{tch}",
                                  tag="ost", bufs=4)
                    nc.vector.tensor_scalar_add(ost[:, :], ps[:, :],
                                                bo_sb[:, fj:fj + 1])
                    nc.sync.dma_start(
                        outT_d[fj * P:(fj + 1) * P, tsl], ost[:, :])

    nc.compile()
    return nc


def prep_inputs(x, k, v, wq, bq, wo, bo):
    x = np.asarray(x, np.float32)
    k = np.asarray(k, np.float32)
    v = np.asarray(v, np.float32)
    wq = np.asarray(wq, np.float32)
    bq = np.asarray(bq, np.float32)
    wo = np.asarray(wo, np.float32)
    bo = np.asarray(bo, np.float32)

    wqT = np.ascontiguousarray((wq * SCALE).T).astype(BF16)
    bqr = np.ascontiguousarray((bq * SCALE).reshape(DT, P).T)
    woT = np.ascontiguousarray(wo.T).astype(BF16)
    bor = np.ascontiguousarray(bo.reshape(DT, P).T)

    in_maps = []
    for b in range(x.shape[0]):
        xT = np.ascontiguousarray(x[b].T).astype(BF16)
        kT = np.zeros((D, S), BF16)
        kT[:, :S0] = k[b].T.astype(BF16)
        vaug = np.zeros((S, H * HW), BF16)
        vb = v[b].astype(BF16)
        for h in range(H):
            vaug[:S0, h * HW:h * HW + Dh] = vb[:, h * Dh:(h + 1) * Dh]
            vaug[:S0, h * HW + Dh] = BF16(1.0)
        in_maps.append({
            "xT": xT, "kT": kT, "vaug": np.ascontiguousarray(vaug),
            "wqT": wqT, "bqr": bqr, "woT": woT, "bor": bor,
        })
    return in_maps


_NC_CACHE = {}


def kernel(x, k, v, wq, bq, wo, bo, _trace=False):
    if "nc" not in _NC_CACHE:
        _NC_CACHE["nc"] = build_bass()
    nc = _NC_CACHE["nc"]
    in_maps = prep_inputs(x, k, v, wq, bq, wo, bo)
    res = bass_utils.run_bass_kernel_spmd(
        nc, in_maps, core_ids=list(range(B)), trace=_trace)
    _NC_CACHE["last_result"] = res
    out = np.stack([np.ascontiguousarray(r["outT"].T) for r in res.results])
    return out
